# revision 1
# baseline (speedup 1.0000x reference)
"""Trainium2 Bass kernel for EnhancedInvariantExtractor.

Input  h [1_000_000, 120] f32:  per atom: 32 scalars | 16 vectors (l=1, dim 3)
                                | 8 tensors (l=2, dim 5).
Output [1_000_000, 204] f32: scalars(32) | vnorm(16) | tnorm(8) | vdots(120)
                             | tdots(28), where vdots/tdots are clipped pairwise
                             cosines of the normalized vectors (upper triangle,
                             row-major).

Strategy (8 NeuronCores, data-parallel over atoms):
- Host transposes each core's shard to feature-major hT [120, 125440] with
  rows reordered [vec(48) | tens(40) | scalars(32)]; engine APs larger than
  32 partitions must start at partition 0, so the hot 88 rows sit first.
  (fp32 can't use the DMA xbar transpose; host transpose is free w.r.t. HW
  exec time.)
- Device output rows are [dots(148) | norms(24) | scalars(32)]; the host
  permutes rows back when assembling (free).
- Device processes chunks of 512 atoms (free dim); features on partitions.
  All per-atom segmented reductions are 0/1-stationary fp16 matmuls on the
  PE (fp32 PSUM accumulation):
    mm1: n2   = S1^T  . X^2     [88 -> 24]   squared norms (4 chunks share
                                             one PSUM bank on 32-row strips)
    mm2: rexp = E4^T  . rinv    [24 -> 88]   1/norm per component row
    mm3: u_k  = P_k^T . vu      [88 -> <=112] pair sums vu_i + vu_j
    mm4: dots = R_k^T . squ_k   [<=112 -> <=32] |u|^2 per pair
  cos(i,j) = |vu_i + vu_j|^2/2 - 1 (self-clipping: |u|^2 >= 0 exactly; the
  upper clip only trims fp rounding above +1).
- Norm path: rinv = exp(-0.5 ln(n2 + eps^2)) on ACT (Rsqrt/Reciprocal are
  banned); norm output = n2 * rinv on DVE. All ACT funcs (Ln, Exp, Square,
  Copy) live in the one 'natural_log_exp_and_others' table set -> no
  LoadActFuncSet churn.
- squ ops process 2 chunks at once ([rk, 1024] PSUM reads) to amortize the
  ScalarE per-op overhead.
"""

import sys

sys.path.insert(0, "/opt/trn_rl_repo")

import numpy as np

N_ATOMS = 1_000_000
N_CORES = 8
PER_CORE = N_ATOMS // N_CORES  # 125_000
CHUNK = 512
N_CHUNKS = 245
PADDED = CHUNK * N_CHUNKS  # 125_440
NF = 120
NR = 128  # device rows: [comps(88) | pad(8) | scalars(32 @ 96)]
NOUT = 204
NOUT_DEV = 212  # device rows: dots(148) pad-aligned | scalars | norms | tail-dots
NV, NT = 16, 8
EPS2 = 1e-12
U_CHUNK_PAIRS = [32, 32, 32, 32, 20]
U_CHUNK_ROWS = [96, 96, 96, 112, 100]

_CACHE = {}


def _vrow(i, d):
    return 3 * i + d


def _trow(t, d):
    return 48 + 5 * t + d


def _pair_list():
    pairs = []
    for i in range(NV):
        for j in range(i + 1, NV):
            pairs.append([(_vrow(i, d), _vrow(j, d)) for d in range(3)])
    for a in range(NT):
        for b in range(a + 1, NT):
            pairs.append([(_trow(a, d), _trow(b, d)) for d in range(5)])
    return pairs


def _stationaries():
    pairs = _pair_list()
    assert len(pairs) == 148

    s1 = np.zeros((88, 24), np.float16)
    for i in range(NV):
        for d in range(3):
            s1[_vrow(i, d), i] = 1.0
    for t in range(NT):
        for d in range(5):
            s1[_trow(t, d), 16 + t] = 1.0

    e4 = np.zeros((120, 88), np.float16)
    for j in range(4):
        e4[32 * j : 32 * j + 24, :] = s1.T

    p_ks, r_ks = [], []
    pbase = 0
    for pk in U_CHUNK_PAIRS:
        chunk_pairs = pairs[pbase : pbase + pk]
        rk = sum(len(c) for c in chunk_pairs)
        p_k = np.zeros((88, rk), np.float16)
        r_k = np.zeros((rk, pk), np.float16)
        r = 0
        for pl, comp in enumerate(chunk_pairs):
            for ri, rj in comp:
                p_k[ri, r] = 1.0
                p_k[rj, r] = 1.0
                r_k[r, pl] = 1.0
                r += 1
        assert r == rk
        p_ks.append(p_k)
        r_ks.append(r_k)
        pbase += pk
    assert [p.shape[1] for p in p_ks] == U_CHUNK_ROWS
    return s1, e4, p_ks, r_ks


def _build_nc(n_chunks=N_CHUNKS, padded=PADDED, reps=1):
    import concourse.bacc as bacc
    import concourse.bass as bass
    import concourse.tile as tile
    from concourse import mybir

    ACT = mybir.ActivationFunctionType
    f32, f16 = mybir.dt.float32, mybir.dt.float16

    import concourse.hw_specs as hw_specs

    if not getattr(hw_specs, "_invx_patched", False):
        _orig_tables = hw_specs.get_activation_tables

        def _only_nle(module_arch):
            tabs = _orig_tables(module_arch)
            keep = "natural_log_exp_and_others"
            assert keep in tabs
            # preserve set indices (walrus maps act_func_set_id by position);
            # empty the other sets so the load-insertion pass can only pick
            # the one covering Ln+Exp+Square+Copy
            return {
                name: (funcs if name == keep else set())
                for name, funcs in tabs.items()
            }

        hw_specs.get_activation_tables = _only_nle
        import concourse.bacc as _bacc_mod

        _bacc_mod.get_activation_tables = _only_nle
        hw_specs._invx_patched = True

    nc = bacc.Bacc("TRN2", target_bir_lowering=False, debug=False, num_devices=N_CORES)

    eps_t = nc.alloc_sbuf_tensor("const-f32-eps2", [128, 1], f32)
    nc.gpsimd.memset(eps_t.ap(), EPS2)
    nc.const_aps.aps[(f32, EPS2)] = eps_t.ap()
    nc.all_engine_barrier()

    ht_ext = nc.declare_dram_parameter("hT", [NR, padded], f32, isOutput=False)
    s1_ext = nc.declare_dram_parameter("S1", [88, 24], f16, isOutput=False)
    e4_ext = nc.declare_dram_parameter("E4", [120, 88], f16, isOutput=False)
    p_exts = [
        nc.declare_dram_parameter(f"P{k}", [88, rk], f16, isOutput=False)
        for k, rk in enumerate(U_CHUNK_ROWS)
    ]
    r_exts = [
        nc.declare_dram_parameter(f"R{k}", [rk, pk], f16, isOutput=False)
        for k, (rk, pk) in enumerate(zip(U_CHUNK_ROWS, U_CHUNK_PAIRS))
    ]
    out_ext = nc.declare_dram_parameter("out", [NOUT_DEV, padded], f32, isOutput=True)

    with tile.TileContext(nc) as tc:
        with (
            tc.tile_pool(name="const", bufs=1) as cpool,
            tc.tile_pool(name="x", bufs=12) as xpool,
            tc.tile_pool(name="sq", bufs=4) as sqpool,
            tc.tile_pool(name="vu", bufs=4) as vupool,
            tc.tile_pool(name="squ", bufs=2) as squpool,
            tc.tile_pool(name="grp", bufs=2) as grppool,
            tc.tile_pool(name="oa", bufs=3) as oapool,
            tc.tile_pool(name="ob", bufs=3) as obpool,
            tc.tile_pool(name="ps_n2", bufs=1, space=bass.MemorySpace.PSUM) as ps_n2,
            tc.tile_pool(name="ps_re", bufs=1, space=bass.MemorySpace.PSUM) as ps_re,
            tc.tile_pool(name="ps_u", bufs=2, space=bass.MemorySpace.PSUM) as ps_u,
            tc.tile_pool(name="ps_d1", bufs=1, space=bass.MemorySpace.PSUM) as ps_d1,
            tc.tile_pool(name="ps_d2", bufs=1, space=bass.MemorySpace.PSUM) as ps_d2,
        ):
            s1_t = cpool.tile([88, 24], f16)
            nc.sync.dma_start(out=s1_t[:], in_=s1_ext[:])
            e4_t = cpool.tile([120, 88], f16)
            nc.sync.dma_start(out=e4_t[:], in_=e4_ext[:])
            p_ts, r_ts = [], []
            for k, rk in enumerate(U_CHUNK_ROWS):
                p_t = cpool.tile([88, rk], f16, tag=f"P{k}")
                nc.sync.dma_start(out=p_t[:], in_=p_exts[k][:])
                p_ts.append(p_t)
                r_t = cpool.tile([rk, U_CHUNK_PAIRS[k]], f16, tag=f"R{k}")
                nc.sync.dma_start(out=r_t[:], in_=r_exts[k][:])
                r_ts.append(r_t)

            for rep in range(reps):
                for g in range(0, n_chunks, 4):
                    chunks = list(range(g, min(g + 4, n_chunks)))

                    # phase A: load, square, per-chunk n2 into strip j
                    n2g = ps_n2.tile([128, CHUNK], f32, tag="n2g")
                    xs, sqs = {}, {}
                    for c in chunks:
                        j = c % 4
                        x_t = xpool.tile([NR, CHUNK], f32, tag="x")
                        nc.sync.dma_start(
                            out=x_t[:], in_=ht_ext[:, c * CHUNK : (c + 1) * CHUNK]
                        )
                        xs[c] = x_t
                        sq_t = sqpool.tile([88, CHUNK], f16, tag="sq")
                        nc.gpsimd.tensor_mul(sq_t[:], x_t[0:88, :], x_t[0:88, :])
                        sqs[c] = sq_t
                        nc.tensor.matmul(
                            n2g[32 * j : 32 * j + 24, :],
                            s1_t[:],
                            sq_t[:],
                            tile_position=(0, 32 * j),
                        )

                    # group norm path (ACT: Ln, Exp; DVE: n2*rinv)
                    lng = grppool.tile([128, CHUNK], f32, tag="lng")
                    nc.scalar.activation(lng[:], n2g[:], ACT.Ln, bias=EPS2, scale=1.0)
                    rinvg = grppool.tile([128, CHUNK], f16, tag="rinvg")
                    nc.scalar.activation(rinvg[:], lng[:], ACT.Exp, bias=0.0, scale=-0.5)
                    normn = grppool.tile([128, CHUNK], f32, tag="normn")
                    nc.vector.tensor_mul(normn[:], n2g[:], rinvg[:])

                    # phase B1: expand rinv, normalize components
                    vus = {}
                    for c in chunks:
                        j = c % 4
                        rexp = ps_re.tile([88, CHUNK], f32, tag="rexp")
                        nc.tensor.matmul(
                            rexp[:],
                            e4_t[32 * j : 32 * j + 24, :],
                            rinvg[32 * j : 32 * j + 24, :],
                            tile_position=(32 * j, 0),
                        )
                        vu_t = vupool.tile([88, CHUNK], f16, tag="vu")
                        nc.vector.tensor_mul(vu_t[:], xs[c][0:88, :], rexp[:])
                        vus[c] = vu_t

                    # phase B2: pair sums + squares, two chunks per squ op
                    squs = {}
                    for p0 in range(0, len(chunks), 2):
                        pc = chunks[p0 : p0 + 2]
                        w = len(pc) * CHUNK
                        for k, rk in enumerate(U_CHUNK_ROWS):
                            u_k = ps_u.tile([rk, 2 * CHUNK], f32, tag="u")
                            for qi, c in enumerate(pc):
                                nc.tensor.matmul(
                                    u_k[:, qi * CHUNK : (qi + 1) * CHUNK],
                                    p_ts[k][:],
                                    vus[c][:],
                                )
                            squ_k = squpool.tile([rk, 2 * CHUNK], f16, tag=f"squ{k}")
                            nc.scalar.activation(
                                squ_k[:, 0:w],
                                u_k[:, 0:w],
                                ACT.Square,
                                bias=0.0,
                                scale=1.0,
                            )
                            squs[(p0 // 2, k)] = squ_k

                    # phase B3: per-chunk dots, output assembly, store
                    for ci, c in enumerate(chunks):
                        j = c % 4
                        qi = ci % 2
                        cols = slice(c * CHUNK, (c + 1) * CHUNK)
                        d1 = ps_d1.tile([128, CHUNK], f32, tag="d1")
                        d2 = ps_d2.tile([20, CHUNK], f32, tag="d2")
                        for k in range(5):
                            squ_k = squs[(ci // 2, k)]
                            rhs = squ_k[:, qi * CHUNK : (qi + 1) * CHUNK]
                            if k < 4:
                                nc.tensor.matmul(
                                    d1[32 * k : 32 * k + 32, :],
                                    r_ts[k][:],
                                    rhs,
                                    tile_position=(0, 32 * k),
                                )
                            else:
                                nc.tensor.matmul(d2[:], r_ts[k][:], rhs)

                        oa = oapool.tile([128, CHUNK], f32, tag="oa")
                        nc.vector.tensor_scalar(
                            oa[:],
                            d1[:],
                            0.5,
                            -1.0,
                            mybir.AluOpType.mult,
                            mybir.AluOpType.add,
                        )
                        ob = obpool.tile([84, CHUNK], f32, tag="ob")
                        nc.gpsimd.tensor_copy(ob[0:32, :], xs[c][96:128, :])
                        nc.vector.tensor_copy(
                            ob[32:64, :], normn[32 * j : 32 * j + 32, :]
                        )
                        nc.vector.tensor_scalar(
                            ob[64:84, :],
                            d2[:],
                            0.5,
                            -1.0,
                            mybir.AluOpType.mult,
                            mybir.AluOpType.add,
                        )

                        nc.sync.dma_start(out=out_ext[0:128, cols], in_=oa[:])
                        nc.sync.dma_start(out=out_ext[128:212, cols], in_=ob[:])

    nc.compile()
    return nc


def _get_nc():
    if "nc" not in _CACHE:
        _CACHE["nc"] = _build_nc()
    return _CACHE["nc"]


def _make_in_map(shard, stat):
    """shard [n<=PADDED, 120] f32 -> feature-major, reordered, padded."""
    buf = np.ones((PADDED, NR), np.float32)
    buf[: shard.shape[0], 0:88] = shard[:, 32:120]
    buf[: shard.shape[0], 96:128] = shard[:, 0:32]
    return {"hT": np.ascontiguousarray(buf.T), **stat}


def _stat_map():
    s1, e4, p_ks, r_ks = _stationaries()
    stat = {"S1": s1, "E4": e4}
    for k in range(5):
        stat[f"P{k}"] = p_ks[k]
        stat[f"R{k}"] = r_ks[k]
    return stat


def _assemble(dev_out, n):
    """dev_out [204, >=n] device layout -> [n, 204] reference layout."""
    o = np.empty((n, NOUT), np.float32)
    o[:, 0:32] = dev_out[128:160, :n].T  # scalars
    o[:, 32:48] = dev_out[160:176, :n].T  # vnorm
    o[:, 48:56] = dev_out[176:184, :n].T  # tnorm
    o[:, 56:176] = dev_out[0:120, :n].T  # vdots
    o[:, 176:184] = dev_out[120:128, :n].T  # tdots: tens pairs 0..7
    o[:, 184:204] = dev_out[192:212, :n].T  # tdots: tens pairs 8..27
    return o


def _run_pjrt(nc, in_maps):
    """Execute the Bass module on N_CORES devices via PJRT/shard_map with
    per-device buffer assembly and per-shard fetch (avoids giant host
    concats, which trip transfer limits on the axon path)."""
    import jax
    from jax.sharding import Mesh, NamedSharding, PartitionSpec
    from jax.experimental.shard_map import shard_map
    from concourse import mybir
    from concourse.bass2jax import (
        _bass_exec_p,
        install_neuronx_cc_hook,
        partition_id_tensor,
    )

    install_neuronx_cc_hook()
    partition_name = nc.partition_id_tensor.name if nc.partition_id_tensor else None
    in_names, out_names, out_avals = [], [], []
    for alloc in nc.m.functions[0].allocations:
        if not isinstance(alloc, mybir.MemoryLocationSet):
            continue
        name = alloc.memorylocations[0].name
        if alloc.kind == "ExternalInput":
            if name != partition_name:
                in_names.append(name)
        elif alloc.kind == "ExternalOutput":
            out_names.append(name)
            shape = tuple(alloc.tensor_shape)
            dtype = mybir.dt.np(alloc.dtype)
            out_avals.append(jax.core.ShapedArray(shape, dtype))
    n_params = len(in_names)
    n_outs = len(out_avals)
    all_in_names = list(in_names) + out_names
    if partition_name is not None:
        all_in_names.append(partition_name)
    donate = tuple(range(n_params, n_params + n_outs))

    def _body(*args):
        operands = list(args)
        if partition_name is not None:
            operands.append(partition_id_tensor())
        outs = _bass_exec_p.bind(
            *operands,
            out_avals=tuple(out_avals),
            in_names=tuple(all_in_names),
            out_names=tuple(out_names),
            lowering_input_output_aliases=(),
            sim_require_finite=True,
            sim_require_nnan=True,
            nc=nc,
        )
        return tuple(outs)

    devices = jax.devices()[:N_CORES]
    mesh = Mesh(np.asarray(devices), ("core",))
    sharding = NamedSharding(mesh, PartitionSpec("core"))
    fn = jax.jit(
        shard_map(
            _body,
            mesh=mesh,
            in_specs=(PartitionSpec("core"),) * (n_params + n_outs),
            out_specs=(PartitionSpec("core"),) * n_outs,
            check_rep=False,
        ),
        donate_argnums=donate,
        keep_unused=True,
    )

    def make_global(per_core_arrays):
        a0 = per_core_arrays[0]
        gshape = (N_CORES * a0.shape[0],) + a0.shape[1:]
        bufs = [
            jax.device_put(per_core_arrays[c], devices[c]) for c in range(N_CORES)
        ]
        return jax.make_array_from_single_device_arrays(gshape, sharding, bufs)

    g_ins = [
        make_global([np.asarray(in_maps[c][nm]) for c in range(N_CORES)])
        for nm in in_names
    ]
    g_zeros = [
        make_global([np.zeros(av.shape, av.dtype) for _ in range(N_CORES)])
        for av in out_avals
    ]
    outs = fn(*g_ins, *g_zeros)
    jax.block_until_ready(outs)

    results = [dict() for _ in range(N_CORES)]
    for i, nm in enumerate(out_names):
        shards = sorted(
            outs[i].addressable_shards, key=lambda s: devices.index(s.device)
        )
        for c, sh in enumerate(shards):
            results[c][nm] = np.asarray(sh.data)
    return results


def kernel(h):
    h = np.asarray(h, dtype=np.float32)
    assert h.shape == (N_ATOMS, NF)

    nc = _get_nc()
    stat = _stat_map()
    in_maps = [
        _make_in_map(h[c * PER_CORE : (c + 1) * PER_CORE], stat)
        for c in range(N_CORES)
    ]
    res = _run_pjrt(nc, in_maps)

    out = np.empty((N_ATOMS, NOUT), np.float32)
    for c in range(N_CORES):
        out[c * PER_CORE : (c + 1) * PER_CORE] = _assemble(res[c]["out"], PER_CORE)
    return out

